# revision 1
# baseline (speedup 1.0000x reference)
"""SupJSD / ContrastiveLossPlus loss kernel for 8 Trainium2 NeuronCores.

Single pass over the [3N, D] data. Per 128-row tile:
  ss_i  = sum_d x^2           (DVE scalar_tensor_tensor, accum)
  s16_i = 16/sqrt(ss)         (ACT: exp(-0.5*ln(ss)+ln16), batched per group)
  lg    = ln(s16*x + 1e-30)   (ACT Ln with per-partition scale)  [= ln(16*p)]
  u_i   = sum_d x*lg          (DVE stt accum, into column 256 of the tile)
  A     = (cls==lab)*s16      (DVE fused tensor_scalar)  [one-hot * 16/||x||]
  psum += A^T @ [x | u]       (PE fp32 matmul, N=257)
Host combines the per-class [80,257] accumulators in float64:
  loss = 0.01/D * sum_c (E'_c - sum_d seg*ln(16*mix)) / counts_c
"""

import numpy as np

N_CORES = 8
N, D, C = 65536, 256, 80
R = 3 * N // N_CORES          # rows per core = 24576
T = R // 128                  # tiles per core = 192
G = 16                        # tiles per small-op group
LOG16 = float(np.log(16.0))

_cache = {}


def _build_nc():
    from contextlib import ExitStack

    import concourse.tile as tile
    from concourse import bacc, mybir

    F32 = mybir.dt.float32
    A = mybir.AluOpType
    ACTF = mybir.ActivationFunctionType

    nc = bacc.Bacc("TRN2", target_bir_lowering=False, debug=False,
                   num_devices=N_CORES)
    xin = nc.dram_tensor("xin", [R, D], F32, kind="ExternalInput").ap()
    labt = nc.dram_tensor("labt", [128, T], F32, kind="ExternalInput").ap()
    cls = nc.dram_tensor("cls", [128, C], F32, kind="ExternalInput").ap()
    out = nc.dram_tensor("acc", [C, D + 1], F32, kind="ExternalOutput").ap()

    with tile.TileContext(nc) as tc, ExitStack() as ctx:
        cpool = ctx.enter_context(tc.tile_pool(name="consts", bufs=1))
        xpool = ctx.enter_context(tc.tile_pool(name="x", bufs=2 * G + 4))
        lgpool = ctx.enter_context(tc.tile_pool(name="lg", bufs=3))
        jpool = ctx.enter_context(tc.tile_pool(name="junk", bufs=2))
        apool = ctx.enter_context(tc.tile_pool(name="amat", bufs=3))
        spool = ctx.enter_context(tc.tile_pool(name="small", bufs=2))
        opool = ctx.enter_context(tc.tile_pool(name="out", bufs=1))
        pspool = ctx.enter_context(tc.tile_pool(name="ps", bufs=1, space="PSUM"))

        clst = cpool.tile([128, C], F32)
        nc.sync.dma_start(clst[:], cls[:])
        labs = cpool.tile([128, T], F32)
        nc.sync.dma_start(labs[:], labt[:])
        c_ln16 = cpool.tile([128, 1], F32)
        nc.vector.memset(c_ln16[:], LOG16)
        c_tiny = cpool.tile([128, 1], F32)
        nc.vector.memset(c_tiny[:], 1e-30)

        ps = pspool.tile([C, D + 1], F32)
        junk1 = jpool.tile([128, D], F32, tag="junk")
        junk2 = jpool.tile([128, D], F32, tag="junk")

        for g in range(T // G):
            xts = []
            ssg = spool.tile([128, G], F32, tag="ssg")
            for j in range(G):
                k = g * G + j
                xu = xpool.tile([128, D + 1], F32, tag="xu")
                nc.sync.dma_start(xu[:, 0:D], xin[k * 128:(k + 1) * 128, :])
                nc.vector.scalar_tensor_tensor(
                    junk1[:], xu[:, 0:D], 1.0, xu[:, 0:D], A.mult, A.mult,
                    accum_out=ssg[:, j:j + 1])
                xts.append(xu)
            # s16 = exp(-0.5*ln(max(ss,1e-24)) + ln16) = 16/sqrt(ss)
            lssg = spool.tile([128, G], F32, tag="lssg")
            nc.vector.tensor_scalar(lssg[:], ssg[:], 1e-24, None, A.max)
            nc.scalar.activation(lssg[:], lssg[:], ACTF.Ln)
            s16g = spool.tile([128, G], F32, tag="s16g")
            nc.scalar.activation(s16g[:], lssg[:], ACTF.Exp,
                                 bias=c_ln16[:], scale=-0.5)
            for j in range(G):
                k = g * G + j
                xu = xts[j]
                s16 = s16g[:, j:j + 1]
                lg = lgpool.tile([128, D], F32, tag="lg")
                nc.scalar.activation(lg[:], xu[:, 0:D], ACTF.Ln,
                                     bias=c_tiny[:], scale=s16)
                nc.vector.scalar_tensor_tensor(
                    junk2[:], xu[:, 0:D], 1.0, lg[:], A.mult, A.mult,
                    accum_out=xu[:, D:D + 1])
                amat = apool.tile([128, C], F32, tag="amat")
                nc.vector.tensor_scalar(amat[:], clst[:], labs[:, k:k + 1],
                                        s16, A.is_equal, A.mult)
                nc.tensor.matmul(ps[:], amat[:], xu[:],
                                 start=(k == 0), stop=(k == T - 1))

        acc = opool.tile([C, D + 1], F32)
        nc.vector.tensor_copy(acc[:], ps[:])
        nc.sync.dma_start(out[:], acc[:])
    nc.compile()
    return nc


def _get_nc():
    if "nc" not in _cache:
        _cache["nc"] = _build_nc()
    return _cache["nc"]


def kernel(logits_clean, logits_aug1, logits_aug2, labels):
    import os

    from concourse.bass_utils import run_bass_kernel_spmd

    x3 = np.concatenate(
        [np.asarray(logits_clean, dtype=np.float32),
         np.asarray(logits_aug1, dtype=np.float32),
         np.asarray(logits_aug2, dtype=np.float32)], axis=0)
    lab1 = np.asarray(labels).astype(np.int64)
    lab3 = np.concatenate([lab1, lab1, lab1])

    cls = np.ascontiguousarray(
        np.broadcast_to(np.arange(C, dtype=np.float32), (128, C)))
    in_maps = []
    for c in range(N_CORES):
        sl = slice(c * R, (c + 1) * R)
        in_maps.append({
            "xin": np.ascontiguousarray(x3[sl]),
            "labt": np.ascontiguousarray(
                lab3[sl].reshape(T, 128).T.astype(np.float32)),
            "cls": cls,
        })

    nc = _get_nc()
    trace = bool(int(os.environ.get("KERNEL_TRACE", "0")))
    kw = {}
    if trace:
        kw = dict(trace=True, tmpdir=os.environ.get("KERNEL_TRACE_DIR"))
    br = run_bass_kernel_spmd(nc, in_maps, list(range(N_CORES)), **kw)
    _cache["last_results"] = br

    acc = np.zeros((C, D + 1), np.float64)
    for c in range(N_CORES):
        acc += br.results[c]["acc"].astype(np.float64)

    seg = acc[:, :D] / 16.0            # sum_{i in c} p_i  (per dim)
    Ep = acc[:, D] / 16.0              # sum_{i in c} sum_d p*ln(16p)
    counts = np.bincount(lab3, minlength=C).astype(np.float64)
    mix = seg / np.maximum(counts, 1.0)[:, None]
    lm16 = np.log(np.maximum(mix, 1e-7)) + np.log(16.0)
    num = Ep - (seg * lm16).sum(1)
    loss = np.where(counts > 0, num / np.maximum(counts, 1.0), 0.0).sum() / D
    return np.float32(0.01 * loss)



# revision 5
# speedup vs baseline: 1.0585x; 1.0585x over previous
"""SupJSD / ContrastiveLossPlus loss kernel for 8 Trainium2 NeuronCores.

v2: fp16 data path, batched ACT Ln, single fp16 matmul per tile.

Per 128-row tile (data arrives fp16, tile-transposed, 16-tile groups):
  ss_k   = sum_d x^2            (DVE stt accum, fp16 in -> fp32 accum)
  s16    = exp(-0.5*ln(ss)+ln16)  (ACT Ln+Exp on [128,16] per group)
  m      = x * s16  (= 16*p)    (DVE tensor_scalar, fp16)
  lnm    = ln(m + 1e-30)        (ACT Ln, BATCHED over the whole group)
  u16_k  = sum_d m*lnm          (DVE stt accum)  [= 16*sum_d p*ln(16p)]
  psum  += onehot_k^T @ m       (PE fp16 matmul, [80,256], onehot from host)
End: e1 = sum_k U[:,k]*W[:,k]   (W = 1/count per row, from host)
Host combines in float64:
  T1 = E1/16 - ln16 * sum_c (sum_d seg)/cnt
  loss = 0.01/D * (T1 - sum_c (1/cnt) sum_d seg*ln(clip(seg/cnt,eps)))
"""

import numpy as np

N_CORES = 8
N, D, C = 65536, 256, 80
R = 3 * N // N_CORES          # rows per core = 24576
T = R // 128                  # tiles per core = 192
G = 16                        # tiles per group (batched Ln)
NG = T // G                   # groups = 12
LOG16 = float(np.log(16.0))

_cache = {}


def _build_nc():
    from contextlib import ExitStack

    import concourse.tile as tile
    from concourse import bacc, mybir

    F32 = mybir.dt.float32
    F16 = mybir.dt.float16
    A = mybir.AluOpType
    ACTF = mybir.ActivationFunctionType

    nc = bacc.Bacc("TRN2", target_bir_lowering=False, debug=False,
                   num_devices=N_CORES)
    xin = nc.dram_tensor("xin", [128, T * D], F16, kind="ExternalInput").ap()
    ohin = nc.dram_tensor("ohin", [128, T * C], F16, kind="ExternalInput").ap()
    win = nc.dram_tensor("win", [128, T], F32, kind="ExternalInput").ap()
    out = nc.dram_tensor("acc", [C, D], F32, kind="ExternalOutput").ap()
    oute = nc.dram_tensor("e1", [128, 1], F32, kind="ExternalOutput").ap()

    with tile.TileContext(nc) as tc, ExitStack() as ctx:
        cpool = ctx.enter_context(tc.tile_pool(name="consts", bufs=1))
        xpool = ctx.enter_context(tc.tile_pool(name="x", bufs=3))
        mpool = ctx.enter_context(tc.tile_pool(name="m", bufs=3))
        lpool = ctx.enter_context(tc.tile_pool(name="lnm", bufs=3))
        opool = ctx.enter_context(tc.tile_pool(name="oh", bufs=3))
        spool = ctx.enter_context(tc.tile_pool(name="small", bufs=3))
        jpool = ctx.enter_context(tc.tile_pool(name="junk", bufs=2))
        rpool = ctx.enter_context(tc.tile_pool(name="res", bufs=1))
        pspool = ctx.enter_context(tc.tile_pool(name="ps", bufs=1, space="PSUM"))

        c_tiny24 = cpool.tile([128, 1], F32)
        nc.vector.memset(c_tiny24[:], 1e-24)
        c_tiny30 = cpool.tile([128, 1], F32)
        nc.vector.memset(c_tiny30[:], 1e-30)
        c_ln16 = cpool.tile([128, 1], F32)
        nc.vector.memset(c_ln16[:], LOG16)

        wt = cpool.tile([128, T], F32)
        nc.sync.dma_start(wt[:], win[:])

        e1cols = cpool.tile([128, NG], F32)
        junk1 = jpool.tile([128, D], F16, tag="junk")
        junk2 = jpool.tile([128, D], F16, tag="junk")
        junkg = jpool.tile([128, G], F32, tag="junkg")
        junkw = jpool.tile([128, NG], F32, tag="junkw")

        ps = pspool.tile([C, D], F32)

        for g in range(NG):
            xg = xpool.tile([128, G * D], F16, tag="xg")
            nc.sync.dma_start(xg[:], xin[:, g * G * D:(g + 1) * G * D])
            ohg = opool.tile([128, G * C], F16, tag="ohg")
            nc.sync.dma_start(ohg[:], ohin[:, g * G * C:(g + 1) * G * C])

            ssg = spool.tile([128, G], F32, tag="ssg")
            for j in range(G):
                nc.vector.scalar_tensor_tensor(
                    junk1[:], xg[:, j * D:(j + 1) * D], 1.0,
                    xg[:, j * D:(j + 1) * D], A.mult, A.mult,
                    accum_out=ssg[:, j:j + 1])
            # s16 = exp(-0.5*ln(ss + 1e-24) + ln16) = 16/sqrt(ss)
            lss = spool.tile([128, G], F32, tag="lss")
            nc.scalar.activation(lss[:], ssg[:], ACTF.Ln, bias=c_tiny24[:])
            s16g = spool.tile([128, G], F32, tag="s16g")
            nc.scalar.activation(s16g[:], lss[:], ACTF.Exp,
                                 bias=c_ln16[:], scale=-0.5)

            mg = mpool.tile([128, G * D], F16, tag="mg")
            for j in range(G):
                nc.vector.tensor_scalar(
                    mg[:, j * D:(j + 1) * D], xg[:, j * D:(j + 1) * D],
                    s16g[:, j:j + 1], None, A.mult)

            lnmg = lpool.tile([128, G * D], F16, tag="lnmg")
            nc.scalar.activation(lnmg[:], mg[:], ACTF.Ln, bias=c_tiny30[:])

            ug = spool.tile([128, G], F32, tag="ug")
            for j in range(G):
                k = g * G + j
                nc.vector.scalar_tensor_tensor(
                    junk2[:], mg[:, j * D:(j + 1) * D], 1.0,
                    lnmg[:, j * D:(j + 1) * D], A.mult, A.mult,
                    accum_out=ug[:, j:j + 1])
                nc.tensor.matmul(ps[:], ohg[:, j * C:(j + 1) * C],
                                 mg[:, j * D:(j + 1) * D],
                                 start=(k == 0), stop=(k == T - 1))
            nc.vector.scalar_tensor_tensor(
                junkg[:], ug[:], 1.0, wt[:, g * G:(g + 1) * G],
                A.mult, A.mult, accum_out=e1cols[:, g:g + 1])

        e1t = rpool.tile([128, 1], F32)
        nc.vector.tensor_scalar(junkw[:], e1cols[:], 1.0, 0.0, A.mult,
                                A.add, accum_out=e1t[:])
        acc = rpool.tile([C, D], F32)
        nc.vector.tensor_copy(acc[:], ps[:])
        nc.sync.dma_start(out[:], acc[:])
        nc.sync.dma_start(oute[:], e1t[:])
    nc.compile()
    return nc


def _get_nc():
    if "nc" not in _cache:
        _cache["nc"] = _build_nc()
    return _cache["nc"]


def kernel(logits_clean, logits_aug1, logits_aug2, labels):
    import os

    from concourse.bass_utils import run_bass_kernel_spmd

    x3 = np.concatenate(
        [np.asarray(logits_clean, dtype=np.float32),
         np.asarray(logits_aug1, dtype=np.float32),
         np.asarray(logits_aug2, dtype=np.float32)], axis=0)
    lab1 = np.asarray(labels).astype(np.int64)
    lab3 = np.concatenate([lab1, lab1, lab1])
    counts = np.bincount(lab3, minlength=C).astype(np.float64)

    # [8, 128, T*D] tile-transposed fp16: partition p of core c holds row
    # c*R + t*128 + p of tile t at columns [t*D, (t+1)*D).
    xt = x3.reshape(N_CORES, T, 128, D).transpose(0, 2, 1, 3)
    xt = np.ascontiguousarray(xt.astype(np.float16).reshape(N_CORES, 128, T * D))
    labt = lab3.reshape(N_CORES, T, 128).transpose(0, 2, 1)  # [8,128,T]
    oh = (labt[..., None] == np.arange(C, dtype=np.int64)).astype(np.float16)
    oh = np.ascontiguousarray(oh.reshape(N_CORES, 128, T * C))
    wrow = (1.0 / np.maximum(counts, 1.0)).astype(np.float32)[labt]  # [8,128,T]
    wrow = np.ascontiguousarray(wrow)

    in_maps = []
    for c in range(N_CORES):
        in_maps.append({"xin": xt[c], "ohin": oh[c], "win": wrow[c]})

    nc = _get_nc()
    trace = bool(int(os.environ.get("KERNEL_TRACE", "0")))
    kw = {}
    if trace:
        kw = dict(trace=True, tmpdir=os.environ.get("KERNEL_TRACE_DIR"))
    br = run_bass_kernel_spmd(nc, in_maps, list(range(N_CORES)), **kw)
    _cache["last_results"] = br

    seg16 = np.zeros((C, D), np.float64)
    E1 = 0.0
    for c in range(N_CORES):
        seg16 += br.results[c]["acc"].astype(np.float64)
        E1 += float(br.results[c]["e1"].astype(np.float64).sum())

    seg = seg16 / 16.0                 # sum_{i in c} p_i (per dim)
    cnt = np.maximum(counts, 1.0)
    mix = seg / cnt[:, None]
    lm = np.log(np.maximum(mix, 1e-7))
    spw = (seg.sum(1) / cnt).sum()     # sum_i w_i * sum_d p_id
    T1 = E1 / 16.0 - LOG16 * spw
    T2 = ((seg * lm).sum(1) / cnt).sum()
    loss = (T1 - T2) / D
    return np.float32(0.01 * loss)
